# revision 1
# baseline (speedup 1.0000x reference)
"""Trainium2 Bass kernel for nn_NeuroKernel_56590489092176.

Math (reference):
    P = N(N+1)/2 upper-tri pairs (x[i], x[j]), j >= i, N = 2048
    h  = sigmoid(pairs @ W1.T + b1)     # [P, 128]
    h  = relu(h @ W2.T + b2)            # [P, 32]
    v  = h @ W3.T + b3                  # [P]
    K  = zeros(N, N); K[triu] = v
    out = K.T @ K

Distribution (8 cores):
    Rows are split into 32 groups of 64 rows. Group g needs col-tiles
    tj in [g//2, 16) (width W_g = 16 - g//2).  Strip k = groups (k, 31-k)
    => exactly 17 col-tile blocks of [64 rows x 128 cols] per strip.
    Core c owns strips 2c and 2c+1 => 34 blocks = 278,528 pairs per core.

    Per core: MLP over its 34 blocks (TensorE matmuls with 32x32 tiling,
    ScalarE sigmoid, VectorE relu), v scattered into a per-core DRAM K
    buffer via indirect DMA (data-driven offsets), strips gathered back,
    masked to the upper triangle, then a partial C_c = sum_s strip_s^T
    strip_s GEMM.  Host sums the 8 partial [2048, 2048] outputs.

Self-contained: hardcodes all shapes; only needs /opt/trn_rl_repo.
"""

import sys

if "/opt/trn_rl_repo" not in sys.path:
    sys.path.insert(0, "/opt/trn_rl_repo")

import numpy as np

import concourse.bass as bass
import concourse.bacc as bacc
import concourse.mybir as mybir
import concourse.tile as tile
from concourse.bass_utils import run_bass_kernel_spmd

N = 2048
NCORES = 8
NBLK = 34          # blocks per core (2 strips x 17)
NRND = NBLK * 4    # 512-pair rounds... (2048-pair rounds actually): 136
F32 = mybir.dt.float32
I32 = mybir.dt.int32
AF = mybir.ActivationFunctionType
ALU = mybir.AluOpType


# ----------------------------------------------------------------- host prep

def _strips_of_core(c):
    """Two strips per core; strip = (groups (k, 31-k), block list)."""
    out = []
    for k in (2 * c, 2 * c + 1):
        blocks = [(k, tj) for tj in range(k // 2, 16)]
        blocks += [(31 - k, tj) for tj in range((31 - k) // 2, 16)]
        assert len(blocks) == 17
        out.append((k, blocks))
    return out


def _host_prep(x, W1, b1, W2, b2, W3, b3):
    """Builds the 8 per-core input maps."""
    x = np.asarray(x, np.float32)
    common = {
        "w1h": np.ascontiguousarray(W1.T.astype(np.float32)),          # [2, 128]
        "b1h": np.ascontiguousarray(b1.astype(np.float32)[:, None]),   # [128, 1]
        "w2h": np.ascontiguousarray(W2.T.astype(np.float32)),          # [128, 32]
        "b2h": np.ascontiguousarray(np.tile(b2.astype(np.float32), 4)[:, None]),
        "w3h": np.ascontiguousarray(np.tile(W3[0].astype(np.float32), 4)[:, None]),
        "b3h": np.full((128, 1), float(b3[0]), np.float32),
    }
    in_maps = []
    for c in range(NCORES):
        strips = _strips_of_core(c)
        pt = np.empty((NBLK, 4, 2, 4, 512), np.float32)
        offsc = np.empty((NRND, 16), np.int32)
        offg = np.empty((2, 128), np.int32)
        kmask = np.empty((2, 128, N), np.float32)
        b = 0
        for s, (k, blocks) in enumerate(strips):
            rows = np.concatenate(
                [64 * k + np.arange(64), 64 * (31 - k) + np.arange(64)]
            ).astype(np.int32)
            offg[s] = rows
            kmask[s] = (np.arange(N)[None, :] >= rows[:, None]).astype(np.float32)
            for grp, tj in blocks:
                xj = np.tile(x[128 * tj : 128 * tj + 128], 4)           # [512]
                for r in range(4):
                    for t in range(4):
                        i0 = 64 * grp + 16 * t + 4 * r
                        pt[b, r, 0, t] = np.repeat(x[i0 : i0 + 4], 128)
                        pt[b, r, 1, t] = xj
                for t in range(4):
                    ii = np.arange(16)
                    i_glob = 64 * grp + 16 * t + ii
                    offsc[4 * b + t] = i_glob * 16 + tj
                b += 1
        assert b == NBLK
        m = dict(common)
        m["pt"] = pt
        m["offsc"] = offsc
        m["offg"] = offg
        m["kmask"] = kmask
        in_maps.append(m)
    return in_maps


# ------------------------------------------------------------- bass program

def build_nc():
    nc = bacc.Bacc("TRN2", target_bir_lowering=False, debug=False)

    ptd = nc.dram_tensor("pt", [NBLK, 4, 2, 4, 512], F32, kind="ExternalInput")
    w1d = nc.dram_tensor("w1h", [2, 128], F32, kind="ExternalInput")
    b1d = nc.dram_tensor("b1h", [128, 1], F32, kind="ExternalInput")
    w2d = nc.dram_tensor("w2h", [128, 32], F32, kind="ExternalInput")
    b2d = nc.dram_tensor("b2h", [128, 1], F32, kind="ExternalInput")
    w3d = nc.dram_tensor("w3h", [128, 1], F32, kind="ExternalInput")
    b3d = nc.dram_tensor("b3h", [128, 1], F32, kind="ExternalInput")
    kbd = nc.dram_tensor("kblk", [NBLK, 64, 128], F32, kind="ExternalOutput")

    with tile.TileContext(nc) as tc:
        with (
            tc.tile_pool(name="consts", bufs=1) as consts,
            tc.tile_pool(name="ptp", bufs=4) as ptp,
            tc.tile_pool(name="h1p", bufs=4) as h1p,
            tc.tile_pool(name="h2p", bufs=3) as h2p,
            tc.tile_pool(name="vp", bufs=3) as vp,
            tc.tile_pool(name="pre1p", bufs=1, space="PSUM") as pre1p,
            tc.tile_pool(name="h2pp", bufs=2, space="PSUM") as h2pp,
            tc.tile_pool(name="vpp", bufs=2, space="PSUM") as vpp,
        ):
            w1sb = consts.tile([128, 128], F32)
            for r in range(4):
                nc.sync.dma_start(w1sb[32 * r : 32 * r + 2, 0:128], w1d.ap())
            w2sb = consts.tile([128, 32], F32)
            nc.sync.dma_start(w2sb[:], w2d.ap())
            w3sb = consts.tile([128, 1], F32)
            nc.sync.dma_start(w3sb[:], w3d.ap())
            b1sb = consts.tile([128, 1], F32)
            nc.sync.dma_start(b1sb[:], b1d.ap())
            b2sb = consts.tile([128, 1], F32)
            nc.sync.dma_start(b2sb[:], b2d.ap())
            b3sb = consts.tile([128, 1], F32)
            nc.sync.dma_start(b3sb[:], b3d.ap())

            # ---------------- MLP over 34 blocks (136 rounds of 2048 pairs)
            # Software-pipelined with 2-round skew so TensorE never waits
            # on ScalarE/VectorE mid-round: iteration i issues
            # L1(i), L2(i-1), L3(i-2) back-to-back on PE.
            nrounds = NBLK * 4
            st = {}  # round index -> per-round tiles

            def stage_l1(i):
                blk, t = divmod(i, 4)
                if t == 0:
                    ptsb = ptp.tile([128, 2048], F32)
                    for r in range(4):
                        nc.sync.dma_start(
                            ptsb[32 * r : 32 * r + 2, 0:2048],
                            ptd.ap()[blk : blk + 1, r : r + 1].rearrange(
                                "a b d t e -> (a b) d (t e)"
                            ).squeeze(0),
                        )
                    st[("pt", blk)] = ptsb
                ptsb = st[("pt", blk)]
                pre1 = pre1p.tile([128, 2048], F32)
                for r in range(4):
                    nc.tensor.matmul(
                        pre1[:, 512 * r : 512 * (r + 1)],
                        lhsT=w1sb[32 * r : 32 * r + 2, 0:128],
                        rhs=ptsb[32 * r : 32 * r + 2, 512 * t : 512 * (t + 1)],
                        start=True,
                        stop=True,
                        tile_position=(32 * r, 0),
                    )
                h1 = h1p.tile([128, 2048], F32)
                nc.scalar.activation(
                    h1[:], pre1[:, 0:2048], AF.Sigmoid, bias=b1sb[:, 0:1], scale=1.0
                )
                st[("h1", i)] = h1

            def stage_l2(i):
                h1 = st.pop(("h1", i))
                h2ps = h2pp.tile([128, 512], F32)
                for cc in range(4):
                    nc.tensor.matmul(
                        h2ps[32 * cc : 32 * cc + 32, 0:512],
                        lhsT=w2sb[:, 0:32],
                        rhs=h1[:, 512 * cc : 512 * (cc + 1)],
                        start=True,
                        stop=True,
                        tile_position=(0, 32 * cc),
                    )
                h2sb = h2p.tile([128, 512], F32)
                nc.vector.tensor_scalar(
                    h2sb[:],
                    h2ps[:, 0:512],
                    scalar1=b2sb[:, 0:1],
                    scalar2=0.0,
                    op0=ALU.add,
                    op1=ALU.max,
                )
                st[("h2", i)] = h2sb

            def stage_l3(i):
                blk, t = divmod(i, 4)
                h2sb = st.pop(("h2", i))
                vps = vpp.tile([128, 512], F32)
                for r in range(4):
                    nc.tensor.matmul(
                        vps[32 * r : 32 * r + 1, 0:512],
                        lhsT=w3sb[32 * r : 32 * r + 32, 0:1],
                        rhs=h2sb[32 * r : 32 * r + 32, 0:512],
                        start=True,
                        stop=True,
                        tile_position=(32 * r, 32 * r),
                    )
                vst = vp.tile([128, 512], F32)
                nc.vector.tensor_scalar(
                    vst[:],
                    vps[:, 0:512],
                    scalar1=b3sb[:, 0:1],
                    scalar2=None,
                    op0=ALU.add,
                )
                v_sb = vst[:].rearrange("(a b) e -> a b e", b=32)[:, 0:1, :]
                nc.gpsimd.dma_start(
                    kbd.ap()[blk : blk + 1, 16 * t : 16 * t + 16, :], v_sb
                )

            for i in range(nrounds + 2):
                if i < nrounds:
                    stage_l1(i)
                if 1 <= i < nrounds + 1:
                    stage_l2(i - 1)
                if i >= 2:
                    stage_l3(i - 2)

    nc.compile()
    return nc


def build_nc_gemm():
    nc = bacc.Bacc("TRN2", target_bir_lowering=False, debug=False)
    ksd = nc.dram_tensor("kst", [2, 128, N], F32, kind="ExternalInput")
    cpd = nc.dram_tensor("cpart", [N, N], F32, kind="ExternalOutput")

    with tile.TileContext(nc) as tc:
        with (
            tc.tile_pool(name="gemm", bufs=1) as gemm,
            tc.tile_pool(name="psp", bufs=2, space="PSUM") as psp,
            tc.tile_pool(name="csbp", bufs=2) as csbp,
        ):
            strips = []
            for s in range(2):
                st = gemm.tile([128, 2048], F32, tag=f"strip{s}")
                nc.sync.dma_start(st[:], ksd.ap()[s : s + 1].squeeze(0))
                strips.append(st)

            for a in range(16):
                cps = psp.tile([128, 2048], F32)
                for j in range(4):
                    nc.tensor.matmul(
                        cps[:, 512 * j : 512 * (j + 1)],
                        lhsT=strips[0][:, 128 * a : 128 * a + 128],
                        rhs=strips[0][:, 512 * j : 512 * (j + 1)],
                        start=True,
                        stop=False,
                    )
                    nc.tensor.matmul(
                        cps[:, 512 * j : 512 * (j + 1)],
                        lhsT=strips[1][:, 128 * a : 128 * a + 128],
                        rhs=strips[1][:, 512 * j : 512 * (j + 1)],
                        start=False,
                        stop=True,
                    )
                csb = csbp.tile([128, 2048], F32)
                nc.vector.tensor_copy(csb[:], cps[:, 0:2048])
                nc.sync.dma_start(cpd.ap()[128 * a : 128 * a + 128, :], csb[:])

    nc.compile()
    return nc


_NC_MLP = None
_NC_GEMM = None

_MLP_INPUTS = ("pt", "w1h", "b1h", "w2h", "b2h", "w3h", "b3h")


def _get_nc():
    global _NC_MLP
    if _NC_MLP is None:
        _NC_MLP = build_nc()
    return _NC_MLP


def _get_nc_gemm():
    global _NC_GEMM
    if _NC_GEMM is None:
        _NC_GEMM = build_nc_gemm()
    return _NC_GEMM


def _assemble_strips(c, kblk, kmask):
    """Host: place a core's 34 v-blocks into its 2 masked K strips."""
    kst = np.zeros((2, 128, N), np.float32)
    b = 0
    for s, (k, blocks) in enumerate(_strips_of_core(c)):
        for grp, tj in blocks:
            half = 0 if grp == k else 1
            kst[s, 64 * half : 64 * half + 64, 128 * tj : 128 * tj + 128] = kblk[b]
            b += 1
    kst *= kmask
    return kst


def kernel(x, W1, b1, W2, b2, W3, b3):
    in_maps = _host_prep(
        np.asarray(x), np.asarray(W1), np.asarray(b1), np.asarray(W2),
        np.asarray(b2), np.asarray(W3), np.asarray(b3),
    )
    mlp_maps = [{k: m[k] for k in _MLP_INPUTS} for m in in_maps]
    res_a = run_bass_kernel_spmd(_get_nc(), mlp_maps, core_ids=list(range(NCORES)))
    gemm_maps = [
        {"kst": _assemble_strips(c, res_a.results[c]["kblk"], in_maps[c]["kmask"])}
        for c in range(NCORES)
    ]
    res_b = run_bass_kernel_spmd(
        _get_nc_gemm(), gemm_maps, core_ids=list(range(NCORES))
    )
    out = np.zeros((N, N), np.float32)
    for c in range(NCORES):
        out += res_b.results[c]["cpart"]
    return out



# revision 13
# speedup vs baseline: 4.4857x; 4.4857x over previous
"""Trainium2 Bass kernel for nn_NeuroKernel_56590489092176.

Math (reference):
    P = N(N+1)/2 upper-tri pairs (x[i], x[j]), j >= i, N = 2048
    h  = sigmoid(pairs @ W1.T + b1)     # [P, 128]
    h  = relu(h @ W2.T + b2)            # [P, 32]
    v  = h @ W3.T + b3                  # [P]
    K  = zeros(N, N); K[triu] = v
    out = K.T @ K

Distribution (8 cores):
    Rows split into 32 groups of 64; strip k = groups (k, 31-k) => 17
    [64 rows x 128 cols] blocks per strip; core c owns strips 2c, 2c+1
    (34 blocks, 278,528 padded pairs per core).

    NEFF 1 (MLP), per core, in 272 subrounds of 1024 pairs:
      L1   TensorE fp32r:  pre1[128f, 1024p] = W1 @ pairs   (1 row/pair)
      sig  ScalarE:        h1 = sigmoid(pre1 + b1) -> bf16
      L2t  TensorE bf16:   z[128p, 32f] = h1_chunk^T @ (W2^T |w3|) + b2|w3|
                           (pairs on PSUM partitions => 0.25 rows/pair;
                            |w3| folded into W2/b2 columns)
      stt  VectorE:        r = max(z, 0) * sign(w3)   (folds W3 + relu)
      red  VectorE:        v[p, c] = sum_f r          (grouped reduce)
      out  v-blocks to DRAM positionally [NBLK, 128, 64] fp32.

    Host: transpose v-blocks into the 2 masked K strips (+b3), fp16.

    NEFF 2 (GEMM), per core: C_c = S0^T S0 + S1^T S1 in fp16 (1 cyc/row),
    fp16 output.  Host sums the 8 partial [2048, 2048] outputs in fp32.

Self-contained: hardcodes all shapes; only needs /opt/trn_rl_repo.
"""

import sys

if "/opt/trn_rl_repo" not in sys.path:
    sys.path.insert(0, "/opt/trn_rl_repo")

import numpy as np

import concourse.bass as bass
import concourse.bacc as bacc
import concourse.mybir as mybir
import concourse.tile as tile
from concourse.bass_utils import run_bass_kernel_spmd

N = 2048
NCORES = 8
NBLK = 34            # blocks per core (2 strips x 17)
NSUB = NBLK * 8      # 1024-pair subrounds: 272
F32 = mybir.dt.float32
F32R = mybir.dt.float32r
BF16 = mybir.dt.bfloat16
F16 = mybir.dt.float16
AF = mybir.ActivationFunctionType
ALU = mybir.AluOpType
AX = mybir.AxisListType


# ----------------------------------------------------------------- host prep

def _strips_of_core(c):
    """Two strips per core; strip = (groups (k, 31-k), block list)."""
    out = []
    for k in (2 * c, 2 * c + 1):
        blocks = [(k, tj) for tj in range(k // 2, 16)]
        blocks += [(31 - k, tj) for tj in range((31 - k) // 2, 16)]
        assert len(blocks) == 17
        out.append((k, blocks))
    return out


def _host_prep(x, W1, b1, W2, b2, W3, b3):
    """Builds the 8 per-core MLP input maps."""
    import ml_dtypes

    bf16 = ml_dtypes.bfloat16
    x = np.asarray(x, np.float32)
    w3 = np.asarray(W3, np.float32)[0]                    # [32]
    aw3 = np.abs(w3)
    common = {
        "w1h": np.ascontiguousarray(W1.T.astype(np.float32)),          # [2, 128]
        "b1h": np.ascontiguousarray(b1.astype(np.float32)[:, None]),   # [128, 1]
        "w2h": np.ascontiguousarray(
            (W2.T.astype(np.float32) * aw3[None, :]).astype(bf16)
        ),                                                              # [128, 32]
        "b2h": np.ascontiguousarray(
            np.tile(b2.astype(np.float32) * aw3, 8)[None, :].astype(bf16)
        ),                                                              # [1, 256]
        "onesh": np.ones((1, 128), bf16),
        "sgnh": np.ascontiguousarray(
            np.broadcast_to(np.tile(np.sign(w3), 16)[None, :], (128, 512))
        ).astype(np.float32),                                           # [128, 512]
    }
    in_maps = []
    for c in range(NCORES):
        strips = _strips_of_core(c)
        pt = np.empty((NBLK, 4, 2, 4, 512), np.float32)
        b = 0
        for s, (k, blocks) in enumerate(strips):
            for grp, tj in blocks:
                xj = np.tile(x[128 * tj : 128 * tj + 128], 4)           # [512]
                for r in range(4):
                    for t in range(4):
                        i0 = 64 * grp + 16 * t + 4 * r
                        pt[b, r, 0, t] = np.repeat(x[i0 : i0 + 4], 128)
                        pt[b, r, 1, t] = xj
                b += 1
        assert b == NBLK
        m = dict(common)
        m["pt"] = pt
        in_maps.append(m)
    return in_maps


# ------------------------------------------------------- NEFF 1: the MLP

def build_nc():
    nc = bacc.Bacc("TRN2", target_bir_lowering=False, debug=False)

    ptd = nc.dram_tensor("pt", [NBLK, 4, 2, 4, 512], F32R, kind="ExternalInput")
    w1d = nc.dram_tensor("w1h", [2, 128], F32R, kind="ExternalInput")
    b1d = nc.dram_tensor("b1h", [128, 1], F32, kind="ExternalInput")
    w2d = nc.dram_tensor("w2h", [128, 32], BF16, kind="ExternalInput")
    b2d = nc.dram_tensor("b2h", [1, 256], BF16, kind="ExternalInput")
    onesd = nc.dram_tensor("onesh", [1, 128], BF16, kind="ExternalInput")
    sgnd = nc.dram_tensor("sgnh", [128, 512], F32, kind="ExternalInput")
    kbd = nc.dram_tensor("kblk", [NBLK, 128, 64], F32, kind="ExternalOutput")

    with tile.TileContext(nc) as tc:
        with (
            tc.tile_pool(name="consts", bufs=1) as consts,
            tc.tile_pool(name="ptp", bufs=3) as ptp,
            tc.tile_pool(name="h1p", bufs=3) as h1p,
            tc.tile_pool(name="rp", bufs=3) as rp,
            tc.tile_pool(name="vp", bufs=3) as vp,
            tc.tile_pool(name="pre1p", bufs=3, space="PSUM") as pre1p,
            tc.tile_pool(name="zpp", bufs=2, space="PSUM") as zpp,
        ):
            w1sb = consts.tile([128, 128], F32R)
            for r in range(4):
                nc.sync.dma_start(w1sb[32 * r : 32 * r + 2, 0:128], w1d.ap())
            b1sb = consts.tile([128, 1], F32)
            nc.sync.dma_start(b1sb[:], b1d.ap())
            w2sb = consts.tile([128, 32], BF16)
            nc.sync.dma_start(w2sb[:], w2d.ap())
            b2sb = consts.tile([1, 256], BF16)
            nc.sync.dma_start(b2sb[:], b2d.ap())
            onesb = consts.tile([1, 128], BF16)
            nc.sync.dma_start(onesb[:], onesd.ap())
            sgnsb = consts.tile([128, 512], F32)
            nc.sync.dma_start(sgnsb[:], sgnd.ap())

            st = {}

            def load_pt_part(blk, r):
                # one [2, 2048] r-group per call; spread across subrounds so
                # the SP queue never backs up behind a burst of 4 issues
                if r == 0:
                    st[("pt", blk)] = ptp.tile([128, 2048], F32R, name="ptsb")
                ptsb = st[("pt", blk)]
                (nc.sync if r < 2 else nc.gpsimd).dma_start(
                    ptsb[32 * r : 32 * r + 2, 0:2048],
                    ptd.ap()[blk : blk + 1, r : r + 1].rearrange(
                        "a b d t e -> (a b) d (t e)"
                    ).squeeze(0),
                )

            def stage_l1(i):
                blk, sub = divmod(i, 8)
                t, h = divmod(sub, 2)
                ptsb = st[("pt", blk)]
                pre1 = pre1p.tile([128, 1024], F32)
                for rho in range(2):
                    r = 2 * h + rho
                    nc.tensor.matmul(
                        pre1[:, 512 * rho : 512 * (rho + 1)],
                        lhsT=w1sb[32 * r : 32 * r + 2, 0:128],
                        rhs=ptsb[32 * r : 32 * r + 2, 512 * t : 512 * (t + 1)],
                        start=True,
                        stop=True,
                        tile_position=(32 * r, 0),
                    )
                st[("pre1", i)] = pre1
                if 2 <= sub < 6 and blk + 1 < NBLK:
                    load_pt_part(blk + 1, sub - 2)  # prefetch next block, 1 DMA/subround
                if sub == 7:
                    st.pop(("pt", blk))

            def stage_sig(i):
                pre1 = st.pop(("pre1", i))
                h1 = h1p.tile([128, 1024], BF16)
                nc.scalar.activation(
                    h1[:], pre1[:, 0:1024], AF.Sigmoid, bias=b1sb[:, 0:1], scale=1.0
                )
                st[("h1", i)] = h1

            def stage_l2(i):
                h1 = st.pop(("h1", i))
                p, odd = divmod(i, 2)
                if odd == 0:
                    st[("z", p)] = zpp.tile([128, 512], F32, name="zps")
                zps = st[("z", p)]
                base = 256 * odd
                nc.tensor.matmul(
                    zps[:, base : base + 256],
                    lhsT=onesb[0:1, 0:128],
                    rhs=b2sb[0:1, 0:256],
                    start=True,
                    stop=False,
                    skip_group_check=True,
                )
                for cc in range(8):
                    nc.tensor.matmul(
                        zps[:, base + 32 * cc : base + 32 * cc + 32],
                        lhsT=h1[:, 128 * cc : 128 * (cc + 1)],
                        rhs=w2sb[:, 0:32],
                        start=False,
                        stop=True,
                        skip_group_check=True,
                    )

            def stage_red(p):
                # one stt+reduce per PAIR of subrounds (i = 2p, 2p+1)
                blk, pr = divmod(p, 4)
                zps = st.pop(("z", p))
                if pr == 0:
                    st[("v", blk)] = vp.tile([128, 64], F32, name="vblk")
                v = st[("v", blk)]
                rsb = rp.tile([128, 512], F32)
                nc.vector.scalar_tensor_tensor(
                    rsb[:],
                    zps[:, 0:512],
                    0.0,
                    sgnsb[:, 0:512],
                    op0=ALU.max,
                    op1=ALU.mult,
                )
                nc.vector.tensor_reduce(
                    v[:, 16 * pr : 16 * pr + 16],
                    rsb[:].rearrange("p (c f) -> p c f", f=32),
                    axis=AX.X,
                    op=ALU.add,
                )
                if pr == 3:
                    v = st.pop(("v", blk))
                    nc.sync.dma_start(kbd.ap()[blk : blk + 1].squeeze(0), v[:])

            for r in range(4):
                load_pt_part(0, r)
            for i in range(NSUB + 3):
                if i < NSUB:
                    stage_l1(i)
                if 1 <= i < NSUB + 1:
                    stage_sig(i - 1)
                if 2 <= i < NSUB + 2:
                    stage_l2(i - 2)
                if i >= 3 and (i - 2) % 2 == 1:
                    stage_red((i - 3) // 2)

    nc.compile()
    return nc


# ------------------------------------------------------- NEFF 2: the GEMM

def build_nc_gemm():
    nc = bacc.Bacc("TRN2", target_bir_lowering=False, debug=False)
    ksd = nc.dram_tensor("kst", [2, 128, N], F16, kind="ExternalInput")
    cpd = nc.dram_tensor("cpart", [N, N], F16, kind="ExternalOutput")

    with tile.TileContext(nc) as tc:
        with (
            tc.tile_pool(name="gemm", bufs=1) as gemm,
            tc.tile_pool(name="psp", bufs=2, space="PSUM") as psp,
            tc.tile_pool(name="csbp", bufs=3) as csbp,
        ):
            strips = []
            for s in range(2):
                stile = gemm.tile([128, 2048], F16, tag=f"strip{s}")
                nc.sync.dma_start(stile[:], ksd.ap()[s : s + 1].squeeze(0))
                strips.append(stile)

            for a in range(16):
                cps = psp.tile([128, 2048], F32)
                for j in range(4):
                    nc.tensor.matmul(
                        cps[:, 512 * j : 512 * (j + 1)],
                        lhsT=strips[0][:, 128 * a : 128 * a + 128],
                        rhs=strips[0][:, 512 * j : 512 * (j + 1)],
                        start=True,
                        stop=False,
                    )
                    nc.tensor.matmul(
                        cps[:, 512 * j : 512 * (j + 1)],
                        lhsT=strips[1][:, 128 * a : 128 * a + 128],
                        rhs=strips[1][:, 512 * j : 512 * (j + 1)],
                        start=False,
                        stop=True,
                    )
                csb = csbp.tile([128, 2048], F16)
                if a % 2 == 0:
                    nc.vector.tensor_copy(csb[:], cps[:, 0:2048])
                else:
                    nc.scalar.copy(csb[:], cps[:, 0:2048])
                nc.sync.dma_start(cpd.ap()[128 * a : 128 * a + 128, :], csb[:])

    nc.compile()
    return nc


_NC_MLP = None
_NC_GEMM = None


def _get_nc():
    global _NC_MLP
    if _NC_MLP is None:
        _NC_MLP = build_nc()
    return _NC_MLP


def _get_nc_gemm():
    global _NC_GEMM
    if _NC_GEMM is None:
        _NC_GEMM = build_nc_gemm()
    return _NC_GEMM


def _assemble_strips(c, kblk, b3):
    """Host: v-blocks [NBLK, 128, 64] -> 2 masked fp16 K strips (+b3).

    v[p, col] of block b holds pair (i = 64*grp + col, j = 128*tj + p);
    the strip row for i is 64*half + col.
    """
    kst = np.zeros((2, 128, N), np.float32)
    b = 0
    for s, (k, blocks) in enumerate(_strips_of_core(c)):
        for grp, tj in blocks:
            half = 0 if grp == k else 1
            kst[s, 64 * half : 64 * half + 64, 128 * tj : 128 * tj + 128] = kblk[b].T
            b += 1
    kst += b3
    for s, k in enumerate((2 * c, 2 * c + 1)):
        rows = np.concatenate(
            [64 * k + np.arange(64), 64 * (31 - k) + np.arange(64)]
        )
        kst[s] *= np.arange(N)[None, :] >= rows[:, None]
    return kst.astype(np.float16)


def kernel(x, W1, b1, W2, b2, W3, b3):
    in_maps = _host_prep(
        np.asarray(x), np.asarray(W1), np.asarray(b1), np.asarray(W2),
        np.asarray(b2), np.asarray(W3), np.asarray(b3),
    )
    res_a = run_bass_kernel_spmd(_get_nc(), in_maps, core_ids=list(range(NCORES)))
    b3f = float(np.asarray(b3, np.float32)[0])
    gemm_maps = [
        {"kst": _assemble_strips(c, res_a.results[c]["kblk"], b3f)}
        for c in range(NCORES)
    ]
    res_b = run_bass_kernel_spmd(
        _get_nc_gemm(), gemm_maps, core_ids=list(range(NCORES))
    )
    out = np.zeros((N, N), np.float32)
    for c in range(NCORES):
        out += res_b.results[c]["cpart"].astype(np.float32)
    return out


# revision 18
# speedup vs baseline: 4.6480x; 1.0362x over previous
"""Trainium2 Bass kernel for nn_NeuroKernel_56590489092176.

Math (reference):
    P = N(N+1)/2 upper-tri pairs (x[i], x[j]), j >= i, N = 2048
    h  = sigmoid(pairs @ W1.T + b1)     # [P, 128]
    h  = relu(h @ W2.T + b2)            # [P, 32]
    v  = h @ W3.T + b3                  # [P]
    K  = zeros(N, N); K[triu] = v
    out = K.T @ K

Distribution (8 cores):
    Rows split into 32 groups of 64; strip k = groups (k, 31-k) => 17
    [64 rows x 128 cols] blocks per strip; core c owns strips 2c, 2c+1
    (34 blocks, 278,528 padded pairs per core).

    NEFF 1 (MLP), per core, in 272 subrounds of 1024 pairs:
      L1   TensorE fp32r:  pre1[128f, 1024p] = W1 @ pairs   (1 row/pair)
      sig  ScalarE:        h1 = sigmoid(pre1 + b1) -> bf16
      L2t  TensorE bf16:   z[128p, 32f] = h1_chunk^T @ (W2^T |w3|) + b2|w3|
                           (pairs on PSUM partitions => 0.25 rows/pair;
                            |w3| folded into W2/b2 columns)
      stt  VectorE:        r = max(z, 0) * sign(w3)   (folds W3 + relu)
      red  VectorE:        v[p, c] = sum_f r          (grouped reduce)
      out  v-blocks to DRAM positionally [NBLK, 128, 64] fp32.

    Host: transpose v-blocks into the 2 masked K strips (+b3), fp16.

    NEFF 2 (GEMM), per core: C_c = S0^T S0 + S1^T S1 in fp16 (1 cyc/row),
    fp16 output.  Host sums the 8 partial [2048, 2048] outputs in fp32.

Self-contained: hardcodes all shapes; only needs /opt/trn_rl_repo.
"""

import sys

if "/opt/trn_rl_repo" not in sys.path:
    sys.path.insert(0, "/opt/trn_rl_repo")

import numpy as np

import concourse.bass as bass
import concourse.bacc as bacc
import concourse.mybir as mybir
import concourse.tile as tile
from concourse.bass_utils import run_bass_kernel_spmd

N = 2048
NCORES = 8
NBLK = 34            # blocks per core (2 strips x 17)
NSUB = NBLK * 8      # 1024-pair subrounds: 272
F32 = mybir.dt.float32
F32R = mybir.dt.float32r
BF16 = mybir.dt.bfloat16
F16 = mybir.dt.float16
AF = mybir.ActivationFunctionType
ALU = mybir.AluOpType
AX = mybir.AxisListType


# ----------------------------------------------------------------- host prep

def _strips_of_core(c):
    """Two strips per core; strip = (groups (k, 31-k), block list)."""
    out = []
    for k in (2 * c, 2 * c + 1):
        blocks = [(k, tj) for tj in range(k // 2, 16)]
        blocks += [(31 - k, tj) for tj in range((31 - k) // 2, 16)]
        assert len(blocks) == 17
        out.append((k, blocks))
    return out


def _host_prep(x, W1, b1, W2, b2, W3, b3):
    """Builds the 8 per-core MLP input maps."""
    import ml_dtypes

    bf16 = ml_dtypes.bfloat16
    x = np.asarray(x, np.float32)
    w3 = np.asarray(W3, np.float32)[0]                    # [32]
    aw3 = np.abs(w3)
    common = {
        "w1h": np.ascontiguousarray(W1.T.astype(np.float32)),          # [2, 128]
        "b1h": np.ascontiguousarray(b1.astype(np.float32)[:, None]),   # [128, 1]
        "w2h": np.ascontiguousarray(
            (W2.T.astype(np.float32) * aw3[None, :]).astype(bf16)
        ),                                                              # [128, 32]
        "b2h": np.ascontiguousarray(
            np.tile(b2.astype(np.float32) * aw3, 8)[None, :].astype(bf16)
        ),                                                              # [1, 256]
        "onesh": np.ones((1, 128), bf16),
        "sgnh": np.ascontiguousarray(
            np.broadcast_to(np.tile(np.sign(w3), 16)[None, :], (128, 512))
        ).astype(np.float32),                                           # [128, 512]
    }
    in_maps = []
    for c in range(NCORES):
        strips = _strips_of_core(c)
        pt = np.empty((NBLK, 4, 2, 4, 512), np.float32)
        b = 0
        for s, (k, blocks) in enumerate(strips):
            for grp, tj in blocks:
                xj = np.tile(x[128 * tj : 128 * tj + 128], 4)           # [512]
                for r in range(4):
                    for t in range(4):
                        i0 = 64 * grp + 16 * t + 4 * r
                        pt[b, r, 0, t] = np.repeat(x[i0 : i0 + 4], 128)
                        pt[b, r, 1, t] = xj
                b += 1
        assert b == NBLK
        m = dict(common)
        m["pt"] = pt
        in_maps.append(m)
    return in_maps


# ------------------------------------------------------- NEFF 1: the MLP

def build_nc():
    nc = bacc.Bacc("TRN2", target_bir_lowering=False, debug=False)

    ptd = nc.dram_tensor("pt", [NBLK, 4, 2, 4, 512], F32R, kind="ExternalInput")
    w1d = nc.dram_tensor("w1h", [2, 128], F32R, kind="ExternalInput")
    b1d = nc.dram_tensor("b1h", [128, 1], F32, kind="ExternalInput")
    w2d = nc.dram_tensor("w2h", [128, 32], BF16, kind="ExternalInput")
    b2d = nc.dram_tensor("b2h", [1, 256], BF16, kind="ExternalInput")
    onesd = nc.dram_tensor("onesh", [1, 128], BF16, kind="ExternalInput")
    sgnd = nc.dram_tensor("sgnh", [128, 512], F32, kind="ExternalInput")
    kbd = nc.dram_tensor("kblk", [NBLK, 128, 64], F32, kind="ExternalOutput")

    with tile.TileContext(nc) as tc:
        with (
            tc.tile_pool(name="consts", bufs=1) as consts,
            tc.tile_pool(name="ptp", bufs=3) as ptp,
            tc.tile_pool(name="h1p", bufs=3) as h1p,
            tc.tile_pool(name="rp", bufs=3) as rp,
            tc.tile_pool(name="vp", bufs=3) as vp,
            tc.tile_pool(name="pre1p", bufs=3, space="PSUM") as pre1p,
            tc.tile_pool(name="zpp", bufs=2, space="PSUM") as zpp,
        ):
            w1sb = consts.tile([128, 128], F32R)
            for r in range(4):
                nc.sync.dma_start(w1sb[32 * r : 32 * r + 2, 0:128], w1d.ap())
            b1sb = consts.tile([128, 1], F32)
            nc.sync.dma_start(b1sb[:], b1d.ap())
            w2sb = consts.tile([128, 32], BF16)
            nc.sync.dma_start(w2sb[:], w2d.ap())
            b2sb = consts.tile([1, 256], BF16)
            nc.sync.dma_start(b2sb[:], b2d.ap())
            onesb = consts.tile([1, 128], BF16)
            nc.sync.dma_start(onesb[:], onesd.ap())
            sgnsb = consts.tile([128, 512], F32)
            nc.sync.dma_start(sgnsb[:], sgnd.ap())

            st = {}

            def load_pt_part(blk, r):
                # one [2, 2048] r-group per call; spread across subrounds so
                # the SP queue never backs up behind a burst of 4 issues
                if r == 0:
                    st[("pt", blk)] = ptp.tile([128, 2048], F32R, name="ptsb")
                ptsb = st[("pt", blk)]
                (nc.sync if r < 2 else nc.gpsimd).dma_start(
                    ptsb[32 * r : 32 * r + 2, 0:2048],
                    ptd.ap()[blk : blk + 1, r : r + 1].rearrange(
                        "a b d t e -> (a b) d (t e)"
                    ).squeeze(0),
                )

            def stage_l1(i):
                blk, sub = divmod(i, 8)
                t, h = divmod(sub, 2)
                ptsb = st[("pt", blk)]
                pre1 = pre1p.tile([128, 1024], F32)
                for rho in range(2):
                    r = 2 * h + rho
                    nc.tensor.matmul(
                        pre1[:, 512 * rho : 512 * (rho + 1)],
                        lhsT=w1sb[32 * r : 32 * r + 2, 0:128],
                        rhs=ptsb[32 * r : 32 * r + 2, 512 * t : 512 * (t + 1)],
                        start=True,
                        stop=True,
                        tile_position=(32 * r, 0),
                    )
                st[("pre1", i)] = pre1
                if 2 <= sub < 6 and blk + 1 < NBLK:
                    load_pt_part(blk + 1, sub - 2)  # prefetch next block, 1 DMA/subround
                if sub == 7:
                    st.pop(("pt", blk))

            def stage_sig(i):
                pre1 = st.pop(("pre1", i))
                h1 = h1p.tile([128, 1024], BF16)
                nc.scalar.activation(
                    h1[:], pre1[:, 0:1024], AF.Sigmoid, bias=b1sb[:, 0:1], scale=1.0
                )
                st[("h1", i)] = h1

            def stage_l2(i):
                h1 = st.pop(("h1", i))
                p, odd = divmod(i, 2)
                if odd == 0:
                    st[("z", p)] = zpp.tile([128, 512], F32, name="zps")
                zps = st[("z", p)]
                base = 256 * odd
                nc.tensor.matmul(
                    zps[:, base : base + 256],
                    lhsT=onesb[0:1, 0:128],
                    rhs=b2sb[0:1, 0:256],
                    start=True,
                    stop=False,
                    skip_group_check=True,
                )
                for cc in range(8):
                    nc.tensor.matmul(
                        zps[:, base + 32 * cc : base + 32 * cc + 32],
                        lhsT=h1[:, 128 * cc : 128 * (cc + 1)],
                        rhs=w2sb[:, 0:32],
                        start=False,
                        stop=True,
                        skip_group_check=True,
                    )

            def stage_red(p):
                # one stt+reduce per PAIR of subrounds (i = 2p, 2p+1)
                blk, pr = divmod(p, 4)
                zps = st.pop(("z", p))
                if pr == 0:
                    st[("v", blk)] = vp.tile([128, 64], F32, name="vblk")
                v = st[("v", blk)]
                rsb = rp.tile([128, 512], F32)
                nc.vector.scalar_tensor_tensor(
                    rsb[:],
                    zps[:, 0:512],
                    0.0,
                    sgnsb[:, 0:512],
                    op0=ALU.max,
                    op1=ALU.mult,
                )
                nc.vector.tensor_reduce(
                    v[:, 16 * pr : 16 * pr + 16],
                    rsb[:].rearrange("p (c f) -> p c f", f=32),
                    axis=AX.X,
                    op=ALU.add,
                )
                if pr == 3:
                    v = st.pop(("v", blk))
                    nc.sync.dma_start(kbd.ap()[blk : blk + 1].squeeze(0), v[:])

            for r in range(4):
                load_pt_part(0, r)
            for i in range(NSUB + 3):
                if i < NSUB:
                    stage_l1(i)
                if 1 <= i < NSUB + 1:
                    stage_sig(i - 1)
                if 2 <= i < NSUB + 2:
                    stage_l2(i - 2)
                if i >= 3 and (i - 2) % 2 == 1:
                    stage_red((i - 3) // 2)

    nc.compile()
    return nc


# ------------------------------------------------------- NEFF 2: the GEMM

def build_nc_gemm():
    """C = S0^T S0 + S1^T S1, upper 512-block-triangle only (C symmetric;
    host mirrors).  Row-tile a covers cols [512*(a//4), 2048)."""
    nc = bacc.Bacc("TRN2", target_bir_lowering=False, debug=False)
    ksd = nc.dram_tensor("kst", [2, 128, N], F16, kind="ExternalInput")
    cpd = nc.dram_tensor("cpart", [N, N], F16, kind="ExternalOutput")

    with tile.TileContext(nc) as tc:
        with (
            tc.tile_pool(name="gemm", bufs=1) as gemm,
            tc.tile_pool(name="psp", bufs=2, space="PSUM") as psp,
            tc.tile_pool(name="csbp", bufs=3) as csbp,
        ):
            warm = gemm.tile([128, 512], F16, tag="warm")
            nc.vector.memset(warm[:], 0.0)
            strips = []
            for s in range(2):
                stile = gemm.tile([128, 2048], F16, tag=f"strip{s}")
                (nc.sync if s == 0 else nc.scalar).dma_start(
                    stile[:], ksd.ap()[s : s + 1].squeeze(0)
                )
                strips.append(stile)
            # ramp the PE p-state while the strip DMAs are in flight
            # (warmup matmuls write into the first C psum tile, overwritten
            # by the real accumulation below)
            cps0 = psp.tile([128, 2048], F32, name="cps")
            for _ in range(8):
                nc.tensor.matmul(
                    cps0[:, 0:512], lhsT=warm[:, 0:128], rhs=warm[:, 0:512],
                    start=True, stop=True, skip_group_check=True,
                )

            for a in range(16):
                j0 = a // 4
                cps = cps0 if a == 0 else psp.tile([128, 2048], F32, name="cps")
                for j in range(j0, 4):
                    nc.tensor.matmul(
                        cps[:, 512 * j : 512 * (j + 1)],
                        lhsT=strips[0][:, 128 * a : 128 * a + 128],
                        rhs=strips[0][:, 512 * j : 512 * (j + 1)],
                        start=True,
                        stop=False,
                        skip_group_check=True,
                    )
                    nc.tensor.matmul(
                        cps[:, 512 * j : 512 * (j + 1)],
                        lhsT=strips[1][:, 128 * a : 128 * a + 128],
                        rhs=strips[1][:, 512 * j : 512 * (j + 1)],
                        start=False,
                        stop=True,
                        skip_group_check=True,
                    )
                w = 2048 - 512 * j0
                csb = csbp.tile([128, 2048], F16)
                if a % 2 == 0:
                    nc.vector.tensor_copy(csb[:, 0:w], cps[:, 512 * j0 : 2048])
                else:
                    nc.scalar.copy(csb[:, 0:w], cps[:, 512 * j0 : 2048])
                nc.sync.dma_start(
                    cpd.ap()[128 * a : 128 * a + 128, 512 * j0 : 2048],
                    csb[:, 0:w],
                )

    nc.compile()
    return nc


_NC_MLP = None
_NC_GEMM = None


def _get_nc():
    global _NC_MLP
    if _NC_MLP is None:
        _NC_MLP = build_nc()
    return _NC_MLP


def _get_nc_gemm():
    global _NC_GEMM
    if _NC_GEMM is None:
        _NC_GEMM = build_nc_gemm()
    return _NC_GEMM


def _assemble_strips(c, kblk, b3):
    """Host: v-blocks [NBLK, 128, 64] -> 2 masked fp16 K strips (+b3).

    v[p, col] of block b holds pair (i = 64*grp + col, j = 128*tj + p);
    the strip row for i is 64*half + col.
    """
    kst = np.zeros((2, 128, N), np.float32)
    b = 0
    for s, (k, blocks) in enumerate(_strips_of_core(c)):
        for grp, tj in blocks:
            half = 0 if grp == k else 1
            kst[s, 64 * half : 64 * half + 64, 128 * tj : 128 * tj + 128] = kblk[b].T
            b += 1
    kst += b3
    for s, k in enumerate((2 * c, 2 * c + 1)):
        rows = np.concatenate(
            [64 * k + np.arange(64), 64 * (31 - k) + np.arange(64)]
        )
        kst[s] *= np.arange(N)[None, :] >= rows[:, None]
    return kst.astype(np.float16)


def kernel(x, W1, b1, W2, b2, W3, b3):
    in_maps = _host_prep(
        np.asarray(x), np.asarray(W1), np.asarray(b1), np.asarray(W2),
        np.asarray(b2), np.asarray(W3), np.asarray(b3),
    )
    res_a = run_bass_kernel_spmd(_get_nc(), in_maps, core_ids=list(range(NCORES)))
    b3f = float(np.asarray(b3, np.float32)[0])
    gemm_maps = [
        {"kst": _assemble_strips(c, res_a.results[c]["kblk"], b3f)}
        for c in range(NCORES)
    ]
    res_b = run_bass_kernel_spmd(
        _get_nc_gemm(), gemm_maps, core_ids=list(range(NCORES))
    )
    out = np.zeros((N, N), np.float32)
    for c in range(NCORES):
        out += res_b.results[c]["cpart"].astype(np.float32)
    # only the upper 512-block-triangle was computed; zero the rest,
    # mirror, and halve the double-counted diagonal 512-blocks
    for bi in range(4):
        out[512 * bi : 512 * (bi + 1), : 512 * bi] = 0.0
    out = out + out.T
    for bi in range(4):
        sl = slice(512 * bi, 512 * (bi + 1))
        out[sl, sl] *= 0.5
    return out
